# revision 1
# baseline (speedup 1.0000x reference)
"""Trainium2 Bass kernel for nn_EnsembleModel (ensemble recommender).

Sharding: data-parallel over the batch axis across 8 NeuronCores
(128 rows each). user_ratings/maps/weights replicated. Per core:
  top branch:  (X @ Wsp) @ Wsd -> top-20 -> map through top_map
  mid branch:  (X @ Wmp) @ Wmd -> top-20 -> map through mid_map
  CF branch:   sim = X @ UnT (UnT = row-normalized user_ratings, transposed,
               precomputed on host; the X-row normalization is a positive
               per-row scale and cannot change that row's ranking, so it is
               skipped), k_full = sim @ R -> top-20.
Top-k on device via DVE max/max_index/match_replace (top-8 per pass),
hierarchical for the 8000/20000-wide branches, index translation through
the scatter maps via per-column indirect DMA gathers.
All matmuls fp32 (rankings need ~1e-6 relative fidelity; fp32r/bf16 fail).
"""
import sys

if "/opt/trn_rl_repo" not in sys.path:
    sys.path.insert(0, "/opt/trn_rl_repo")

import numpy as np
from contextlib import ExitStack

import concourse.bass as bass
import concourse.bacc as bacc
import concourse.mybir as mybir
from concourse.tile import TileContext
from concourse.bass_utils import run_bass_kernel_spmd

P = 128
B, N, N_TOP, N_MID, D, N_USERS, K = 1024, 20000, 2000, 8000, 256, 2000, 20
N_CORES = 8
B_LOC = B // N_CORES          # 128
NT = (N + P - 1) // P         # 157 k-tiles over items; last tile K=32
K_LAST = N - (NT - 1) * P     # 32
NTI = NT * P                  # 20096 padded item dim
UT = (N_USERS + P - 1) // P   # 16 u-tiles; last K=80
U_LAST = N_USERS - (UT - 1) * P
CH = 500                      # top-k chunk width
NCH_CF = N // CH              # 40
NCH_MID = N_MID // CH         # 16
NCH_TOP = N_TOP // CH         # 4
C_MID = NCH_MID * 8           # 128 candidate cols
C_CF = NCH_CF * 8             # 320 candidate cols
NEG = -1e30
OFF = 1.0e6                   # min-trick offset (exact in fp32)

F32 = mybir.dt.float32
U32 = mybir.dt.uint32
I32 = mybir.dt.int32


def _u_chunks():
    # sim moving-operand chunks over the 2000 users
    out, off = [], 0
    while off < N_USERS:
        w = min(512, N_USERS - off)
        out.append((off, w))
        off += w
    return out


def build_program():
    nc = bacc.Bacc(None, target_bir_lowering=False)

    xt = nc.dram_tensor("xt", [P, NTI], F32, kind="ExternalInput")
    unt = nc.dram_tensor("unt", [N, N_USERS], F32, kind="ExternalInput")
    rmat = nc.dram_tensor("rmat", [N_USERS, N], F32, kind="ExternalInput")
    wsp = nc.dram_tensor("wsp", [N, D], F32, kind="ExternalInput")
    wmp = nc.dram_tensor("wmp", [N, D], F32, kind="ExternalInput")
    wsd = nc.dram_tensor("wsd", [D, N_TOP], F32, kind="ExternalInput")
    wmd = nc.dram_tensor("wmd", [D, N_MID], F32, kind="ExternalInput")
    tmap = nc.dram_tensor("tmap", [N_TOP, 1], I32, kind="ExternalInput")
    mmap = nc.dram_tensor("mmap", [N_MID, 1], I32, kind="ExternalInput")
    ident_d = nc.dram_tensor("ident", [P, P], F32, kind="ExternalInput")
    cb_mid_d = nc.dram_tensor("cb_mid", [P, C_MID], F32, kind="ExternalInput")
    cb_cf_d = nc.dram_tensor("cb_cf", [P, C_CF], F32, kind="ExternalInput")
    out_d = nc.dram_tensor("out", [P, 3, K], I32, kind="ExternalOutput")

    with TileContext(nc) as tc, ExitStack() as ctx:
        sb = ctx.enter_context(tc.tile_pool(name="sb", bufs=1))
        w_pool = ctx.enter_context(tc.tile_pool(name="wp", bufs=8))
        wd_pool = ctx.enter_context(tc.tile_pool(name="wd", bufs=4))
        unt_pool = ctx.enter_context(tc.tile_pool(name="up", bufs=16))
        r_pool = ctx.enter_context(tc.tile_pool(name="rp", bufs=12))
        scr = ctx.enter_context(tc.tile_pool(name="scr", bufs=2))
        pp_sim = ctx.enter_context(tc.tile_pool(name="pps", bufs=4, space="PSUM"))
        pp_chunk = ctx.enter_context(tc.tile_pool(name="ppc", bufs=2, space="PSUM"))
        pp_hid = ctx.enter_context(tc.tile_pool(name="pph", bufs=2, space="PSUM"))

        # ---------------- constants + XT staging ----------------
        ident = sb.tile([P, P], F32, tag="ident")
        nc.sync.dma_start(out=ident[:], in_=ident_d[:, :])
        cb_mid = sb.tile([P, C_MID], F32, tag="cbm")
        nc.sync.dma_start(out=cb_mid[:], in_=cb_mid_d[:, :])
        cb_cf = sb.tile([P, C_CF], F32, tag="cbc")
        nc.sync.dma_start(out=cb_cf[:], in_=cb_cf_d[:, :])

        xt_sb = sb.tile([P, NTI], F32, tag="xt")
        q = NTI // 4
        for i in range(4):
            nc.sync.dma_start(out=xt_sb[:, i * q:(i + 1) * q],
                              in_=xt[:, i * q:(i + 1) * q])

        def kk(t):
            return P if t < NT - 1 else K_LAST

        # ---------------- fused: sim = X @ UnT AND priors in one k-loop ----------------
        uch = _u_chunks()
        sim_ps = []
        for _uc in range(len(uch)):
            sim_ps_t = pp_sim.tile([P, 512], F32, tag="sim")
            sim_ps.append(sim_ps_t)
        hid_ps = {}
        for bkey in ("s", "m"):
            hid_ps_t = pp_hid.tile([P, 512], F32, tag="hid")
            hid_ps[bkey] = hid_ps_t
        for t in range(NT):
            k = kk(t)
            for ci, (uo, uw) in enumerate(uch):
                u_t = unt_pool.tile([P, 512], F32, tag="unt")
                nc.sync.dma_start(out=u_t[0:k, 0:uw],
                                  in_=unt[t * P:t * P + k, uo:uo + uw])
                nc.tensor.matmul(sim_ps[ci][:, 0:uw],
                                 lhsT=xt_sb[0:k, t * P:(t + 1) * P],
                                 rhs=u_t[0:k, 0:uw],
                                 start=(t == 0), stop=(t == NT - 1))
            for bkey, w_d in (("s", wsp), ("m", wmp)):
                w_t = w_pool.tile([P, D], F32, tag="w")
                nc.sync.dma_start(out=w_t[0:k, :], in_=w_d[t * P:t * P + k, :])
                nc.tensor.matmul(hid_ps[bkey][:, 0:D],
                                 lhsT=xt_sb[0:k, t * P:(t + 1) * P],
                                 rhs=w_t[0:k, :],
                                 start=(t == 0), stop=(t == NT - 1))
        hidT = {}
        for bkey in ("s", "m"):
            hid_sb = sb.tile([P, D], F32, tag=f"hid{bkey}")
            nc.vector.tensor_copy(hid_sb[:], hid_ps[bkey][:, 0:D])
            ht = sb.tile([P, 2 * P], F32, tag=f"hidT{bkey}")
            for dt in range(2):
                tp_ps = pp_chunk.tile([P, 512], F32, tag="pchunk")
                nc.tensor.transpose(out=tp_ps[:, 0:P],
                                    in_=hid_sb[:, dt * P:(dt + 1) * P],
                                    identity=ident[:])
                nc.vector.tensor_copy(ht[:, dt * P:(dt + 1) * P], tp_ps[:, 0:P])
            hidT[bkey] = ht

        sim_sb = sb.tile([P, N_USERS], F32, tag="simsb")
        for ci, (uo, uw) in enumerate(uch):
            nc.vector.tensor_copy(sim_sb[:, uo:uo + uw], sim_ps[ci][:, 0:uw])

        # simT via PE transposes
        simT = sb.tile([P, UT * P], F32, tag="simT")
        for j in range(UT):
            uw = P if j < UT - 1 else U_LAST
            tp_ps = pp_chunk.tile([P, 512], F32, tag="pchunk")
            nc.tensor.transpose(out=tp_ps[0:uw, 0:P],
                                in_=sim_sb[:, j * P:j * P + uw],
                                identity=ident[:])
            nc.vector.tensor_copy(simT[0:uw, j * P:(j + 1) * P], tp_ps[0:uw, 0:P])

        # ---------------- top decoder -> SBUF, direct top-k ----------------
        top_sb = sb.tile([P, N_TOP], F32, tag="topsb")
        for c in range(NCH_TOP):
            ps = pp_chunk.tile([P, 512], F32, tag="pchunk")
            for dt in range(2):
                wt = wd_pool.tile([P, CH], F32, tag="wdec")
                nc.sync.dma_start(out=wt[:, :],
                                  in_=wsd[dt * P:(dt + 1) * P, c * CH:(c + 1) * CH])
                nc.tensor.matmul(ps[:, 0:CH], lhsT=hidT["s"][:, dt * P:(dt + 1) * P],
                                 rhs=wt[:, :], start=(dt == 0), stop=(dt == 1))
            nc.vector.tensor_copy(top_sb[:, c * CH:(c + 1) * CH], ps[:, 0:CH])

        top_idx = sb.tile([P, 24], U32, tag="topidx")
        tv8 = scr.tile([P, 8], F32, tag="v8")
        for r in range(3):
            nc.vector.max(out=tv8[:], in_=top_sb[:])
            nc.vector.max_index(out=top_idx[:, r * 8:(r + 1) * 8],
                                in_max=tv8[:], in_values=top_sb[:])
            if r < 2:
                nc.vector.match_replace(out=top_sb[:], in_to_replace=tv8[:],
                                        in_values=top_sb[:], imm_value=NEG)
            if r < 2:
                tv8 = scr.tile([P, 8], F32, tag="v8")

        top_out = sb.tile([P, K], I32, tag="topout")
        for j in range(K):
            nc.gpsimd.indirect_dma_start(
                out=top_out[:, j:j + 1], out_offset=None, in_=tmap[:, :],
                in_offset=bass.IndirectOffsetOnAxis(ap=top_idx[:, j:j + 1], axis=0))
        nc.sync.dma_start(out=out_d[:, 0, :], in_=top_out[:])

        # ---------------- helpers for chunked branches ----------------
        def l2_extract(cand_vals, cand_idx_u, cb_tile, C, out_name):
            """3 rounds of top-8 over the candidate set; per extracted value,
            one-hot match against the pristine candidate values and min-reduce
            the (global index - OFF) payload. Returns [P, K] f32 global
            indices (+OFF already re-added)."""
            gidx = sb.tile([P, C], F32, tag=f"gidx{out_name}")
            nc.vector.tensor_copy(gidx[:], cand_idx_u[:])          # u32 -> f32
            nc.vector.tensor_tensor(out=gidx[:], in0=gidx[:], in1=cb_tile[:],
                                    op=mybir.AluOpType.add)        # + (base - OFF)
            work = sb.tile([P, C], F32, tag=f"work{out_name}")
            nc.vector.tensor_copy(work[:], cand_vals[:])
            pidx = sb.tile([P, K], F32, tag=f"pidx{out_name}")
            for r in range(3):
                v8 = scr.tile([P, 8], F32, tag="v8l2")
                nc.vector.max(out=v8[:], in_=work[:])
                njj = 8 if r < 2 else K - 16
                for jj in range(njj):
                    j = r * 8 + jj
                    eqm = scr.tile([P, C], F32, tag=f"eq{out_name}")
                    nc.vector.tensor_tensor(out=eqm[:], in0=cand_vals[:],
                                            in1=v8[:, jj:jj + 1].to_broadcast([P, C]),
                                            op=mybir.AluOpType.is_equal)
                    nc.vector.tensor_tensor(out=eqm[:], in0=eqm[:], in1=gidx[:],
                                            op=mybir.AluOpType.mult)
                    nc.vector.tensor_reduce(out=pidx[:, j:j + 1], in_=eqm[:],
                                            axis=mybir.AxisListType.X,
                                            op=mybir.AluOpType.min)
                if r < 2:
                    nc.vector.match_replace(out=work[:], in_to_replace=v8[:],
                                            in_values=work[:], imm_value=NEG)
            nc.vector.tensor_scalar_add(pidx[:], pidx[:], OFF)
            return pidx

        # ---------------- mid decoder, chunked top-k ----------------
        cand_vals_m = sb.tile([P, C_MID], F32, tag="cvm")
        cand_idx_m = sb.tile([P, C_MID], U32, tag="cim")
        for c in range(NCH_MID):
            ps = pp_chunk.tile([P, 512], F32, tag="pchunk")
            for dt in range(2):
                wt = wd_pool.tile([P, CH], F32, tag="wdec")
                nc.sync.dma_start(out=wt[:, :],
                                  in_=wmd[dt * P:(dt + 1) * P, c * CH:(c + 1) * CH])
                nc.tensor.matmul(ps[:, 0:CH], lhsT=hidT["m"][:, dt * P:(dt + 1) * P],
                                 rhs=wt[:, :], start=(dt == 0), stop=(dt == 1))
            nc.vector.max(out=cand_vals_m[:, c * 8:(c + 1) * 8], in_=ps[:, 0:CH])
            nc.vector.max_index(out=cand_idx_m[:, c * 8:(c + 1) * 8],
                                in_max=cand_vals_m[:, c * 8:(c + 1) * 8],
                                in_values=ps[:, 0:CH])

        pidx_m = l2_extract(cand_vals_m, cand_idx_m, cb_mid, C_MID, "m")
        pidx_m_u = sb.tile([P, K], U32, tag="pmu")
        nc.vector.tensor_copy(pidx_m_u[:], pidx_m[:])
        mid_out = sb.tile([P, K], I32, tag="midout")
        for j in range(K):
            nc.gpsimd.indirect_dma_start(
                out=mid_out[:, j:j + 1], out_offset=None, in_=mmap[:, :],
                in_offset=bass.IndirectOffsetOnAxis(ap=pidx_m_u[:, j:j + 1], axis=0))
        nc.sync.dma_start(out=out_d[:, 1, :], in_=mid_out[:])

        # ---------------- k_full = sim @ R, chunked top-k ----------------
        cand_vals_c = sb.tile([P, C_CF], F32, tag="cvc")
        cand_idx_c = sb.tile([P, C_CF], U32, tag="cic")
        for c in range(NCH_CF):
            ps = pp_chunk.tile([P, 512], F32, tag="pchunk")
            for j in range(UT):
                uw = P if j < UT - 1 else U_LAST
                r_t = r_pool.tile([P, CH], F32, tag="r")
                nc.sync.dma_start(out=r_t[0:uw, :],
                                  in_=rmat[j * P:j * P + uw, c * CH:(c + 1) * CH])
                nc.tensor.matmul(ps[:, 0:CH],
                                 lhsT=simT[0:uw, j * P:(j + 1) * P],
                                 rhs=r_t[0:uw, :],
                                 start=(j == 0), stop=(j == UT - 1))
            nc.vector.max(out=cand_vals_c[:, c * 8:(c + 1) * 8], in_=ps[:, 0:CH])
            nc.vector.max_index(out=cand_idx_c[:, c * 8:(c + 1) * 8],
                                in_max=cand_vals_c[:, c * 8:(c + 1) * 8],
                                in_values=ps[:, 0:CH])

        pidx_c = l2_extract(cand_vals_c, cand_idx_c, cb_cf, C_CF, "c")
        cf_out = sb.tile([P, K], I32, tag="cfout")
        nc.vector.tensor_copy(cf_out[:], pidx_c[:])
        nc.sync.dma_start(out=out_d[:, 2, :], in_=cf_out[:])

    nc.compile()
    return nc


_NC_CACHE = None


def _get_program():
    global _NC_CACHE
    if _NC_CACHE is None:
        _NC_CACHE = build_program()
    return _NC_CACHE


def kernel(X, mask, top_map, mid_map, user_ratings, user_personalities,
           Wsp, bsp, Wsd, bsd, Wmp, bmp, Wmd, bmd, k, **_unused):
    assert int(k) == K
    X = np.ascontiguousarray(np.asarray(X, np.float32))
    R = np.ascontiguousarray(np.asarray(user_ratings, np.float32))
    Wsp = np.ascontiguousarray(np.asarray(Wsp, np.float32))
    Wmp = np.ascontiguousarray(np.asarray(Wmp, np.float32))
    Wsd = np.ascontiguousarray(np.asarray(Wsd, np.float32))
    Wmd = np.ascontiguousarray(np.asarray(Wmd, np.float32))
    top_map = np.asarray(top_map, np.int32).reshape(N_TOP, 1)
    mid_map = np.asarray(mid_map, np.int32).reshape(N_MID, 1)

    # Un exactly as the reference computes it (fp32 elementwise)
    norms = np.linalg.norm(R, axis=1).astype(np.float32)
    Un = R / (norms[:, None] + np.float32(1e-8))
    UnT = np.ascontiguousarray(Un.T)

    ident = np.eye(P, dtype=np.float32)
    cb_mid = np.broadcast_to(
        (np.repeat(np.arange(NCH_MID, dtype=np.float32) * CH, 8) - np.float32(OFF)),
        (P, C_MID)).copy()
    cb_cf = np.broadcast_to(
        (np.repeat(np.arange(NCH_CF, dtype=np.float32) * CH, 8) - np.float32(OFF)),
        (P, C_CF)).copy()

    in_maps = []
    for c in range(N_CORES):
        xs = X[c * B_LOC:(c + 1) * B_LOC]                       # [128, 20000]
        xpad = np.zeros((B_LOC, NTI), np.float32)
        xpad[:, :N] = xs
        # [p, t, b] = X[b, t*128+p]
        xt_im = np.ascontiguousarray(
            xpad.reshape(B_LOC, NT, P).transpose(2, 1, 0).reshape(P, NTI))
        in_maps.append(dict(xt=xt_im, unt=UnT, rmat=R, wsp=Wsp, wmp=Wmp,
                            wsd=Wsd, wmd=Wmd, tmap=top_map, mmap=mid_map,
                            ident=ident, cb_mid=cb_mid, cb_cf=cb_cf))

    nc = _get_program()
    res = run_bass_kernel_spmd(nc, in_maps, core_ids=list(range(N_CORES)))
    out = np.concatenate([r["out"] for r in res.results], axis=0)
    return out.astype(np.int32)

